# revision 1
# baseline (speedup 1.0000x reference)
"""Trainium2 Bass kernel for CustomCombinedLoss (weighted BCE sum + MultiMarginLoss).

loss = -sum(w * (pos_t*log(p) + (1-pos_t)*log(1-p)))          # w=2 for target==0
     + sum_{i: target_i>0} (1/C) * sum_{j != y_i} max(0, margin - x[i,y_i] + x[i,j])

Sharding: pure data parallel over the batch dim, B=16384 rows -> 8 cores x 2048 rows.
Each core computes a partial scalar loss; host sums the 8 partials.

Key optimizations over the f32 baseline (61 us):
  - predictions are downcast to fp16 on the host: halves HBM traffic (the
    bottleneck).  Margin-term error from fp16 quantization is ~1e-6 relative,
    far inside the 2e-2 gate; the BCE side stays f32 end to end.
  - xy = pred[r, y_r] extraction no longer burns a full-tile DVE pass per tile
    (2.2 us each).  Per tile, a gpsimd ap_gather (~0.4 us, otherwise-idle
    engine) fetches 16 candidate f16 pairs per partition (each 16-partition
    group shares its rows' y>>1 indices); a [128,32] scalar_tensor_tensor
    picks the right pair slot + parity (~0.3 us on DVE).
  - the hinge is split across ACT and DVE so both engines finish with the DMA:
    ACT tiles:  activation(Relu, bias=margin-xy, accum_out)    ~2.3 us eff
    DVE tiles:  tensor_scalar(max, add) cache-reduce where
                relu(x+b) = max(x, -b) + b and the reduce seed scalar2 = C*b
                makes accum = sum_j relu(x_j + b) directly     ~2.5 us eff
    (HW: the DVE cache-reduce variant always runs 1x; 2x/4x packed modes
    exist only without accum, measured 812 ns vs 2400 ns.)
  - predictions DMA as [128, 8KB/partition] supertiles (two row tiles
    column-paired by the host) for descriptor-optimal streaming.

Layout: row r = g*128 + p of the shard lives at partition p, tile g (0..15).
pred DRAM is [NSUP=8, P=128, 2*C] fp16; supertile s holds tiles 2s, 2s+1.
"""

from contextlib import ExitStack

import numpy as np

import concourse.bacc as bacc
import concourse.bass as bass
import concourse.mybir as mybir
import concourse.tile as tile
from concourse.bass_utils import run_bass_kernel_spmd

WEIGHT = 2.0
MARGIN = 0.5
B, C = 16384, 2048
NCORES = 8
BS = B // NCORES          # rows per core
P = 128                   # partitions
T = BS // P               # row tiles per core
SUPT = 2                  # tiles per streamed supertile
NSUP = T // SUPT
NPAIR = C // 2            # f16 pairs per row
F32 = mybir.dt.float32
F16 = mybir.dt.float16
I16 = mybir.dt.int16

AluOp = mybir.AluOpType
ActFn = mybir.ActivationFunctionType
AxisList = mybir.AxisListType

# three hinge paths, balanced to measured effective per-tile costs:
#   ACT  (~3.0 us): activation(Relu, bias, accum_out) -> per-row sums
#   PE   (~2.8 us PE + 0.75 us DVE): DVE 4x relu junk = max(x, nb2), PE
#        column-sum matmuls accumulate into PSUM; corrections in epilogue
#   CR   (~2.7 us): DVE tensor_scalar(max, add) cache-reduce -> per-row sums
# per-supertile scalar prep runs on the otherwise-idle gpsimd.
PE_TILES = frozenset({1, 3, 5, 7, 9, 11, 13, 14})
CR_TILES = frozenset()


def _loss_program(nc: bass.Bass, tc: "tile.TileContext", pred, pcrit, paux, out):
    ctx = ExitStack()
    with ctx:
        const_pool = ctx.enter_context(tc.tile_pool(name="const", bufs=1))
        small_pool = ctx.enter_context(tc.tile_pool(name="small", bufs=1))
        pred_pool = ctx.enter_context(tc.tile_pool(name="pred", bufs=12))

        # ---- warmup: dummy ap_gather on memset tiles so the gpsimd custom-op
        # library IRAM load (~6 us one-time) overlaps the stream head instead
        # of stalling the first real gather.
        wsrc = const_pool.tile([P, 32], F16)
        nc.gpsimd.memset(wsrc[:], 0.0)
        widx = const_pool.tile([P, 1], I16)
        nc.gpsimd.memset(widx[:], 0)
        wout = const_pool.tile([P, 32], F16)
        nc.gpsimd.ap_gather(
            wout[:], wsrc[:].rearrange("p (n d) -> p n d", d=2), widx[:],
            channels=P, num_elems=16, d=2, num_idxs=16,
        )

        # ---- critical smalls (posm f16 | yidx bits | iota32) first on the
        # SYNC queue, one 128B/partition DMA, so the xy chain unblocks with
        # supertile 0.  pprob/tgt feed only the epilogue -> ACT HWDGE ring.
        crit_t = small_pool.tile([P, 2 * T + 32], F16)
        nc.sync.dma_start(crit_t[:], pcrit[:])
        posm_t = crit_t[:, 0:T]
        yidx_t = crit_t[:, T : 2 * T].bitcast(I16)
        iota32 = crit_t[:, 2 * T : 2 * T + 32]
        aux_t = small_pool.tile([P, 2 * T], F32)
        nc.scalar.dma_start(aux_t[:], paux[:])
        pprob_t = aux_t[:, 0:T]
        tgt_t = aux_t[:, T : 2 * T]

        # scratch outputs (never read); one per engine so ACT/DVE don't
        # serialize on a shared WAW hazard
        junk_dve = const_pool.tile([P, C], F16)
        junk_act = const_pool.tile([P, C], F16)
        junk32 = const_pool.tile([P, 32], F16)

        # gathered candidate pairs, [128, 16 slots * 2] per tile
        apo = const_pool.tile([P, T * 32], F16)
        # per-row hinge params; filled per supertile as xy becomes known
        xy_t = small_pool.tile([P, T], F32)
        negb_t = small_pool.tile([P, T], F32)   # xy - margin
        bias_t = small_pool.tile([P, T], F32)   # margin - xy          (ACT bias)
        nb2_t = small_pool.tile([P, T], F32)    # pos?(xy-margin):16   (DVE relu s1)
        cb_t = small_pool.tile([P, T], F32)     # C*(margin-xy)        (CR s2)
        acc_t = small_pool.tile([P, T], F32)    # ACT cols: sum_j relu; PE cols: C*bias
        nc.vector.memset(nb2_t[:], 0.0)
        ones16 = small_pool.tile([P, 1], F16)
        nc.vector.memset(ones16[:], 1.0)
        junk_pe0 = const_pool.tile([P, C], F16)
        junk_pe1 = const_pool.tile([P, C], F16)
        t1 = small_pool.tile([P, 1], F32)
        t2 = small_pool.tile([P, 1], F32)

        # ---- BCE row terms (all [P, T] f32, off the critical path)
        pos_t = small_pool.tile([P, T], F32)
        nc.vector.tensor_scalar(pos_t[:], tgt_t, 1.0, None, AluOp.min)
        q_t = small_pool.tile([P, T], F32)
        nc.vector.tensor_scalar(q_t[:], pprob_t, -1.0, 1.0, AluOp.mult, AluOp.add)
        lp_t = small_pool.tile([P, T], F32)
        nc.scalar.activation(lp_t[:], pprob_t, ActFn.Ln)
        lq_t = small_pool.tile([P, T], F32)
        nc.scalar.activation(lq_t[:], q_t[:], ActFn.Ln)
        nc.vector.tensor_scalar(lp_t[:], lp_t[:], -100.0, None, AluOp.max)
        nc.vector.tensor_scalar(lq_t[:], lq_t[:], -100.0, None, AluOp.max)

        # row_total = pos_t*(acc/C - lp - MARGIN/C) + (2*pos_t - 2)*lq
        lp2_t = small_pool.tile([P, T], F32)
        nc.vector.tensor_scalar(lp2_t[:], lp_t[:], MARGIN / C, None, AluOp.add)
        c2_t = small_pool.tile([P, T], F32)
        nc.vector.tensor_scalar(c2_t[:], pos_t[:], 2.0, -2.0, AluOp.mult, AluOp.add)
        d_t = small_pool.tile([P, T], F32)
        nc.vector.tensor_mul(d_t[:], c2_t[:], lq_t[:])
        inv_c_t = small_pool.tile([P, 1], F32)
        nc.vector.memset(inv_c_t[:], 1.0 / C)
        ones_t = small_pool.tile([P, 1], F32)
        nc.vector.memset(ones_t[:], 1.0)

        # PSUM accumulator for PE-path column sums
        psum_pool = ctx.enter_context(tc.tile_pool(name="psum", bufs=1, space="PSUM"))
        colsum_ps = psum_pool.tile([1, 4 * 512], F32)

        # ---- stream supertiles; per tile: gather xy, then hinge path
        for s in range(NSUP):
            st = pred_pool.tile([P, SUPT * C], F16, tag="pred")
            nc.sync.dma_start(st[:], pred[s])
            g0 = s * SUPT
            for b in range(SUPT):
                g = g0 + b
                blk = st[:, b * C : (b + 1) * C]
                # 16 candidate pairs/partition (group-shared y>>1 indices)
                pairs = blk.rearrange("p (n d) -> p n d", d=2)
                nc.gpsimd.ap_gather(
                    apo[:, g * 32 : (g + 1) * 32], pairs,
                    yidx_t[:, g : g + 1], channels=P, num_elems=NPAIR, d=2,
                    num_idxs=16,
                )
                # xy = pair value at slot (p%16)*2 + (y&1)
                nc.vector.scalar_tensor_tensor(
                    junk32[:], iota32, posm_t[:, g : g + 1],
                    apo[:, g * 32 : (g + 1) * 32],
                    AluOp.is_equal, AluOp.mult, accum_out=xy_t[:, g : g + 1],
                )
            cols = slice(g0, g0 + SUPT)
            nc.vector.tensor_scalar(
                negb_t[:, cols], xy_t[:, cols], -MARGIN, None, AluOp.add
            )
            nc.vector.tensor_scalar(
                bias_t[:, cols], negb_t[:, cols], -1.0, None, AluOp.mult
            )
            for b in range(SUPT):
                g = g0 + b
                gg = slice(g, g + 1)
                if g in PE_TILES:
                    # acc placeholder C*bias (epilogue: a = acc/C-lp2 = bias-lp2)
                    nc.vector.tensor_scalar(
                        acc_t[:, gg], negb_t[:, gg], -float(C), None, AluOp.mult
                    )
                    # nb2: pos rows -> negb, neg rows -> exactly 16.0 (the PE
                    # colsum picks up an exactly-cancellable 16/neg-row)
                    nc.vector.tensor_scalar(t1[:], negb_t[:, gg], -16.0, None, AluOp.add)
                    nc.vector.tensor_tensor(t2[:], pos_t[:, gg], t1[:], AluOp.mult)
                    nc.vector.tensor_scalar(nb2_t[:, gg], t2[:], 16.0, None, AluOp.add)
                elif g in CR_TILES:
                    nc.vector.tensor_scalar(
                        cb_t[:, gg], negb_t[:, gg], -float(C), None, AluOp.mult
                    )
            jpe = junk_pe0 if s % 2 == 0 else junk_pe1
            for b in range(SUPT):
                g = g0 + b
                gg = slice(g, g + 1)
                blk = st[:, b * C : (b + 1) * C]
                if g in PE_TILES:
                    nc.vector.tensor_scalar(
                        jpe[:], blk, nb2_t[:, gg], None, AluOp.max
                    )
                    first, last = g == min(PE_TILES), g == max(PE_TILES)
                    for k in range(4):
                        csl = slice(k * 512, (k + 1) * 512)
                        nc.tensor.matmul(
                            colsum_ps[:, csl], ones16[:], jpe[:, csl],
                            start=first, stop=last,
                        )
                elif g in CR_TILES:
                    nc.vector.tensor_scalar(
                        junk_dve[:], blk, negb_t[:, gg], cb_t[:, gg],
                        AluOp.max, AluOp.add, accum_out=acc_t[:, gg],
                    )
                else:
                    nc.scalar.activation(
                        junk_act[:], blk, ActFn.Relu, bias=bias_t[:, gg],
                        scale=1.0, accum_out=acc_t[:, gg],
                    )

        # ---- epilogue
        # a = acc/C - lp2: ACT cols -> (sum relu)/C - lp2; PE cols -> bias - lp2
        rowred = small_pool.tile([P, 1], F32)
        a_t = small_pool.tile([P, T], F32)
        nc.vector.scalar_tensor_tensor(
            a_t[:], acc_t[:], inv_c_t[:, 0:1], lp2_t[:],
            AluOp.mult, AluOp.subtract,
        )
        b_t = small_pool.tile([P, T], F32)
        nc.vector.tensor_mul(b_t[:], pos_t[:], a_t[:])
        e_t = small_pool.tile([P, T], F32)
        nc.vector.tensor_add(e_t[:], b_t[:], d_t[:])
        # PE-col correction: -(1-pos)*nb2 = +0.5*c2*nb2 (nb2 is 0 on ACT cols)
        w_t = small_pool.tile([P, T], F32)
        nc.vector.tensor_mul(w_t[:], c2_t[:], nb2_t[:])
        e2_t = small_pool.tile([P, T], F32)
        nc.vector.scalar_tensor_tensor(
            e2_t[:], w_t[:], 0.5, e_t[:], AluOp.mult, AluOp.add
        )
        nc.vector.reduce_sum(rowred[:], e2_t[:], axis=AxisList.X)
        # cross-partition sum via PE: ones[128,1].T @ rowred[128,1] -> [1,1]
        total_ps = psum_pool.tile([1, 1], F32)
        nc.tensor.matmul(total_ps[:], rowred[:], ones_t[:], start=True, stop=True)
        total = small_pool.tile([1, 1], F32)
        nc.vector.tensor_copy(total[:], total_ps[:])
        # PE colsum chunks -> scalars (2 on ACT, 2 on DVE), then /C into total
        sc = small_pool.tile([1, 4], F32)
        jr = small_pool.tile([1, 512], F32)
        jr2 = small_pool.tile([1, 512], F32)
        for k in range(4):
            csl = slice(k * 512, (k + 1) * 512)
            if k < 2:
                nc.scalar.activation(
                    jr[:], colsum_ps[:, csl], ActFn.Copy,
                    accum_out=sc[:, k : k + 1],
                )
            else:
                nc.vector.reduce_sum(
                    sc[:, k : k + 1], colsum_ps[:, csl], axis=AxisList.X
                )
        s01 = small_pool.tile([1, 1], F32)
        nc.vector.tensor_add(s01[:], sc[:, 0:1], sc[:, 1:2])
        s23 = small_pool.tile([1, 1], F32)
        nc.vector.tensor_add(s23[:], sc[:, 2:3], sc[:, 3:4])
        s03 = small_pool.tile([1, 1], F32)
        nc.vector.tensor_add(s03[:], s01[:], s23[:])
        total2 = small_pool.tile([1, 1], F32)
        nc.vector.scalar_tensor_tensor(
            total2[:], s03[:], 1.0 / C, total[:], AluOp.mult, AluOp.add
        )
        nc.sync.dma_start(out[:], total2[:])


def build_nc() -> bass.Bass:
    nc = bacc.Bacc("TRN2", target_bir_lowering=False, debug=False, num_devices=NCORES)
    pred = nc.dram_tensor("pred", [NSUP, P, SUPT * C], F16, kind="ExternalInput").ap()
    pcrit = nc.dram_tensor("pcrit", [P, 2 * T + 32], F16, kind="ExternalInput").ap()
    paux = nc.dram_tensor("paux", [P, 2 * T], F32, kind="ExternalInput").ap()
    out = nc.dram_tensor("out", [1, 1], F32, kind="ExternalOutput").ap()
    with tile.TileContext(nc) as tc:
        _loss_program(nc, tc, pred, pcrit, paux, out)
    nc.compile()
    return nc


def make_in_maps(positive_prob, predictions, target):
    """Shard full inputs into per-core input maps (host-side reshapes only)."""
    pp_all = np.asarray(positive_prob, dtype=np.float32)
    tg_all = np.asarray(target).astype(np.int64)
    pr_all = np.asarray(predictions, dtype=np.float32)
    prow = np.arange(P, dtype=np.int64) % 16
    in_maps = []
    for i in range(NCORES):
        sl = slice(i * BS, (i + 1) * BS)
        # [BS] -> [P, T]: row g*P + p lands at [p, g], matching the row tiling
        pp = np.ascontiguousarray(pp_all[sl].reshape(T, P).T)
        tg = tg_all[sl]
        tgf = np.ascontiguousarray(tg.astype(np.float32).reshape(T, P).T)
        y = np.maximum(tg - 1, 0)
        # per-row pair index (y>>1) and STT compare slot (p%16)*2 + (y&1)
        yidx = np.ascontiguousarray((y >> 1).astype(np.int16).reshape(T, P).T)
        ymod2 = (y & 1).reshape(T, P).T          # [P, T]
        posm = (prow[:, None] * 2 + ymod2).astype(np.float16)
        iota32 = np.broadcast_to(np.arange(32, dtype=np.float16), (P, 32))
        pcrit = np.ascontiguousarray(
            np.concatenate([posm, yidx.view(np.float16), iota32], axis=1)
        )
        paux = np.ascontiguousarray(
            np.concatenate([pp, tgf], axis=1).astype(np.float32)
        )
        # supertile layout: [NSUP, P, 2*C], tiles 2s,2s+1 column-paired so each
        # partition row is 8KB contiguous (descriptor-optimal DMA)
        pr16 = pr_all[sl].astype(np.float16).reshape(NSUP, SUPT, P, C)
        pr16 = np.ascontiguousarray(pr16.transpose(0, 2, 1, 3).reshape(NSUP, P, SUPT * C))
        in_maps.append({"pred": pr16, "pcrit": pcrit, "paux": paux})
    return in_maps


_NC_CACHE = []


def kernel(positive_prob, predictions, target):
    in_maps = make_in_maps(positive_prob, predictions, target)
    if not _NC_CACHE:
        _NC_CACHE.append(build_nc())
    nc = _NC_CACHE[0]
    res = run_bass_kernel_spmd(nc, in_maps, list(range(NCORES)))
    total = np.float32(0.0)
    for r in res.results:
        total += np.float32(r["out"][0, 0])
    return np.asarray(total, dtype=np.float32)



# revision 4
# speedup vs baseline: 1.5581x; 1.5581x over previous
"""Trainium2 Bass kernel for CustomCombinedLoss (weighted BCE sum + MultiMarginLoss).

loss = -sum(w * (pos_t*log(p) + (1-pos_t)*log(1-p)))          # w=2 for target==0
     + sum_{i: target_i>0} (1/C) * sum_{j != y_i} max(0, margin - x[i,y_i] + x[i,j])

Sharding: pure data parallel over the batch dim, B=16384 rows -> 8 cores x 2048 rows.
Each core computes a partial scalar loss; host sums the 8 partials.

v2 design (from the 43-49us fp16 baseline):
  - predictions stream as fp8 e3m4 (range +-15.5, 4-bit mantissa): halves HBM
    traffic to ~4.2 MB/core, a ~12.5 us DMA floor at ~340 GB/s.  Margin-term
    error from fp8 quantization is ~1e-4 relative (vs the 2e-2 gate).
  - all O(B) per-row parameters (pos, 2*pos-2, margin-xy biases, the PE-path
    max() thresholds and their corrections) are host-precomputed and arrive in
    one small f32 side DMA.  This removes the gpsimd ap_gather xy-extraction
    entirely - whose one-time ~6us Q7 IRAM library load was the baseline's
    critical-path bottleneck (first hinge op could not start until ~16us).
  - hinge work split across three engine paths, balanced to measured fp8
    per-tile costs ([128,2048] tile):
      ACT  (~2.19us): activation(Relu, bias=margin-xy, accum_out) -> row sums
      PE   (~1.13us DVE + ~1.0-1.8us PE): DVE tensor_scalar max -> fp8 junk,
           PE ones-stationary matmuls column-sum the junk into one [1,512]
           PSUM bank accumulated across all PE tiles; per-row corrections are
           host-precomputed constants.  The max() threshold is pre-rounded to
           fp8 on the host so the junk output cast is exact.
      CR   (tail only): DVE tensor_scalar(max, add) cache-reduce, where
           accum = scalar2_seed + sum_j max(x_j, scalar1)  (seed semantics).
    The last tile is column-split ACT|CR so both engines finish ~1.2us after
    the final (quarter-size) DMA chunk lands.
  - per-row loss assembled on DVE over a [128, 17] grid (16 tiles + the
    split-CR slot), reduced cross-partition via a tiny f32 matmul; the PE
    colsum bank is reduced by one ACT copy-accum.

Layout: row r = g*128 + p of the shard lives at partition p, grid column g.
pred DRAM is [128, 16*2048] fp8: tile g at columns [g*2048, (g+1)*2048).
"""

from contextlib import ExitStack

import ml_dtypes
import numpy as np

import concourse.bacc as bacc
import concourse.bass as bass
import concourse.mybir as mybir
import concourse.tile as tile
from concourse.bass_utils import run_bass_kernel_spmd

WEIGHT = 2.0
MARGIN = 0.5
B, C = 16384, 2048
NCORES = 8
BS = B // NCORES          # rows per core
P = 128                   # partitions
T = BS // P               # row tiles per core (16)
TX = T + 1                # grid cols incl. the split-CR accumulator slot
F32 = mybir.dt.float32
F16 = mybir.dt.float16
F8 = mybir.dt.float8e3
NPF8 = ml_dtypes.float8_e3m4

AluOp = mybir.AluOpType
ActFn = mybir.ActivationFunctionType
AxisList = mybir.AxisListType

# tile -> engine path assignment (tunable)
ACT_TILES = frozenset({0, 3, 6, 9, 12})
PE_TILES = frozenset({1, 2, 4, 5, 7, 8, 10, 11, 13, 14})
CR_TILES = frozenset()
SPLIT_TILE = 15
SPLIT_COL = 1280          # [0:SPLIT_COL] -> ACT, [SPLIT_COL:C] -> CR
NCR = C - SPLIT_COL

# DMA chunk schedule: (first_tile, n_tiles); small head for fast pipeline
# start, small tail to shorten the post-stream compute tail.
CHUNKS = ((0, 1), (1, 1), (2, 2), (4, 2), (6, 2), (8, 2), (10, 2), (12, 2),
          (14, 1), (15, 1))

# paux column offsets (all f32)
O_PPROB = 0               # [T]
O_POS = O_PPROB + T       # [TX]
O_C2 = O_POS + TX         # [T]
O_BIASA = O_C2 + T        # [T]
O_NB2 = O_BIASA + T       # [T]
O_ACC = O_NB2 + T         # [TX]
O_H3 = O_ACC + TX         # [TX]
O_S1CR = O_H3 + TX        # [1]
O_S2CR = O_S1CR + 1       # [1]
NAUX = O_S2CR + 1


def _loss_program(nc: bass.Bass, tc: "tile.TileContext", pred, paux, out):
    ctx = ExitStack()
    with ctx:
        small_pool = ctx.enter_context(tc.tile_pool(name="small", bufs=1))
        pred_pool = ctx.enter_context(tc.tile_pool(name="pred", bufs=len(CHUNKS)))
        psum_pool = ctx.enter_context(tc.tile_pool(name="psum", bufs=1, space="PSUM"))

        aux_t = small_pool.tile([P, NAUX], F32)
        nc.scalar.dma_start(aux_t[:], paux[:])
        pprob = aux_t[:, O_PPROB : O_PPROB + T]
        pos_x = aux_t[:, O_POS : O_POS + TX]
        c2 = aux_t[:, O_C2 : O_C2 + T]
        acc = aux_t[:, O_ACC : O_ACC + TX]
        h3_x = aux_t[:, O_H3 : O_H3 + TX]

        ones8 = small_pool.tile([P, 1], F8)
        nc.vector.memset(ones8[:], 1.0)
        ones_t = small_pool.tile([P, 1], F32)
        nc.vector.memset(ones_t[:], 1.0)
        lp_x = small_pool.tile([P, TX], F32)
        nc.vector.memset(lp_x[:], 0.0)
        dh_x = small_pool.tile([P, TX], F32)
        nc.vector.memset(dh_x[:], 0.0)

        # ---- BCE row terms (off the critical path; Ln first so the ACT
        # table set containing Ln loads once, early)
        nc.scalar.activation(lp_x[:, 0:T], pprob, ActFn.Ln)
        q_t = small_pool.tile([P, T], F32)
        nc.vector.tensor_scalar(q_t[:], pprob, -1.0, 1.0, AluOp.mult, AluOp.add)
        lq_t = small_pool.tile([P, T], F32)
        nc.scalar.activation(lq_t[:], q_t[:], ActFn.Ln)
        d_t = small_pool.tile([P, T], F32)
        nc.vector.tensor_mul(d_t[:], c2, lq_t[:])
        # dh = c2*lq + h3  (col T stays h3's 0)
        nc.vector.tensor_add(dh_x[:, 0:T], d_t[:], h3_x[:, 0:T])
        nc.vector.tensor_copy(dh_x[:, T:TX], h3_x[:, T:TX])

        # scratch outputs (never read)
        jact = small_pool.tile([P, C], F8)
        jcr = small_pool.tile([P, NCR], F16)
        jpes = [
            small_pool.tile([P, C], F8, name=f"jpe{i}") for i in range(3)
        ]

        colsum_ps = psum_pool.tile([1, 512], F32)
        first_pe = min(PE_TILES)
        last_pe = max(PE_TILES)

        # ---- stream chunks
        npe = 0
        for g0, ntiles in CHUNKS:
            st = pred_pool.tile([P, 2 * C], F8, tag="pred")
            nc.sync.dma_start(
                st[:, 0 : ntiles * C], pred[:, g0 * C : (g0 + ntiles) * C]
            )
            for b in range(ntiles):
                g = g0 + b
                blk = st[:, b * C : (b + 1) * C]
                gg = slice(g, g + 1)
                if g in ACT_TILES:
                    nc.scalar.activation(
                        jact[:], blk, ActFn.Relu,
                        bias=aux_t[:, O_BIASA + g : O_BIASA + g + 1],
                        scale=1.0, accum_out=acc[:, gg],
                    )
                elif g in PE_TILES:
                    jpe = jpes[npe % 3]
                    npe += 1
                    nc.vector.tensor_scalar(
                        jpe[:], blk, aux_t[:, O_NB2 + g : O_NB2 + g + 1],
                        None, AluOp.max,
                    )
                    for k in range(4):
                        csl = slice(k * 512, (k + 1) * 512)
                        nc.tensor.matmul(
                            colsum_ps[:], ones8[:], jpe[:, csl],
                            start=(g == first_pe and k == 0),
                            stop=(g == last_pe and k == 3),
                        )
                elif g in CR_TILES:
                    nc.vector.tensor_scalar(
                        jcr[:, 0:NCR], blk[:, 0:NCR],
                        aux_t[:, O_S1CR : O_S1CR + 1],
                        aux_t[:, O_S2CR : O_S2CR + 1],
                        AluOp.max, AluOp.add, accum_out=acc[:, gg],
                    )
                else:  # SPLIT_TILE
                    nc.scalar.activation(
                        jact[:, 0:SPLIT_COL], blk[:, 0:SPLIT_COL], ActFn.Relu,
                        bias=aux_t[:, O_BIASA + g : O_BIASA + g + 1],
                        scale=1.0, accum_out=acc[:, gg],
                    )
                    nc.vector.tensor_scalar(
                        jcr[:], blk[:, SPLIT_COL:C],
                        aux_t[:, O_S1CR : O_S1CR + 1],
                        aux_t[:, O_S2CR : O_S2CR + 1],
                        AluOp.max, AluOp.add, accum_out=acc[:, T : T + 1],
                    )

        # ---- epilogue
        # row_total = pos*(acc/C - lp) + (c2*lq + h3); h3 already folds the
        # -pos*M/C self-term and the PE-path neg-row correction.
        a_x = small_pool.tile([P, TX], F32)
        nc.vector.scalar_tensor_tensor(
            a_x[:], acc, 1.0 / C, lp_x[:], AluOp.mult, AluOp.subtract
        )
        b2_x = small_pool.tile([P, TX], F32)
        nc.vector.tensor_mul(b2_x[:], pos_x, a_x[:])
        e2_x = small_pool.tile([P, TX], F32)
        nc.vector.tensor_add(e2_x[:], b2_x[:], dh_x[:])
        rowred = small_pool.tile([P, 1], F32)
        nc.vector.reduce_sum(rowred[:], e2_x[:], axis=AxisList.X)
        # cross-partition sum via PE: rowred[128,1].T @ ones[128,1] -> [1,1]
        total_ps = psum_pool.tile([1, 1], F32)
        nc.tensor.matmul(total_ps[:], rowred[:], ones_t[:], start=True, stop=True)
        # PE colsum bank -> scalar on ACT (copy with accumulate)
        cs_junk = small_pool.tile([1, 512], F32)
        cs_sc = small_pool.tile([1, 1], F32)
        nc.scalar.activation(
            cs_junk[:], colsum_ps[:], ActFn.Copy, accum_out=cs_sc[:]
        )
        total = small_pool.tile([1, 1], F32)
        nc.vector.tensor_copy(total[:], total_ps[:])
        total2 = small_pool.tile([1, 1], F32)
        nc.vector.scalar_tensor_tensor(
            total2[:], cs_sc[:], 1.0 / C, total[:], AluOp.mult, AluOp.add
        )
        nc.sync.dma_start(out[:], total2[:])


def build_nc() -> bass.Bass:
    nc = bacc.Bacc("TRN2", target_bir_lowering=False, debug=False, num_devices=NCORES)
    pred = nc.dram_tensor("pred", [P, T * C], F8, kind="ExternalInput").ap()
    paux = nc.dram_tensor("paux", [P, NAUX], F32, kind="ExternalInput").ap()
    out = nc.dram_tensor("out", [1, 1], F32, kind="ExternalOutput").ap()
    with tile.TileContext(nc) as tc:
        _loss_program(nc, tc, pred, paux, out)
    nc.compile()
    return nc


def make_in_maps(positive_prob, predictions, target):
    """Shard full inputs into per-core input maps (host-side prep only)."""
    pp_all = np.asarray(positive_prob, dtype=np.float32)
    tg_all = np.asarray(target).astype(np.int64)
    pr_all = np.asarray(predictions, dtype=np.float32)
    M = np.float32(MARGIN)
    in_maps = []
    for i in range(NCORES):
        sl = slice(i * BS, (i + 1) * BS)
        pr = pr_all[sl]                                   # [BS, C] f32
        pr8 = pr.astype(NPF8)
        # [BS, C] -> [P, T*C]: row g*P + p -> partition p, cols [g*C,(g+1)*C)
        pred8 = np.ascontiguousarray(
            pr8.reshape(T, P, C).transpose(1, 0, 2).reshape(P, T * C)
        )
        pp = pp_all[sl].reshape(T, P).T                   # [P, T]
        tg = tg_all[sl]
        pos = (tg != 0).astype(np.float32).reshape(T, P).T
        c2 = 2.0 * pos - 2.0
        y = np.maximum(tg - 1, 0)
        xy = pr[np.arange(BS), y].reshape(T, P).T         # exact f32 [P, T]
        biasA = (M - xy).astype(np.float32)
        # PE path: threshold nb2 pre-rounded to fp8 (so the fp8 junk cast is
        # exact); 8.0 for negative rows (all junk elems become exactly 8.0,
        # the largest power of two under the e3m4 max of 15.5).
        nb2q = (xy - M).astype(NPF8).astype(np.float32)
        nb2_all = np.where(pos > 0, nb2q, np.float32(8.0))
        pe_mask = np.zeros((1, T), dtype=np.float32)
        for g in PE_TILES:
            pe_mask[0, g] = 1.0
        nb2 = nb2_all * pe_mask
        acc0 = np.zeros((P, TX), dtype=np.float32)
        acc0[:, 0:T] = -np.float32(C) * nb2
        # h3 = 0.5*c2*nb2 - pos*M/C  (PE neg-row cancel + margin self-term)
        h3 = np.zeros((P, TX), dtype=np.float32)
        h3[:, 0:T] = 0.5 * c2 * nb2 - pos * (M / np.float32(C))
        pos_ext = np.concatenate([pos, pos[:, SPLIT_TILE : SPLIT_TILE + 1]], axis=1)
        s1cr = (xy - M)[:, SPLIT_TILE : SPLIT_TILE + 1].astype(np.float32)
        s2cr = np.float32(NCR) * (M - xy)[:, SPLIT_TILE : SPLIT_TILE + 1]
        paux = np.ascontiguousarray(
            np.concatenate(
                [pp, pos_ext, c2, biasA, nb2, acc0, h3, s1cr, s2cr], axis=1
            ).astype(np.float32)
        )
        in_maps.append({"pred": pred8, "paux": paux})
    return in_maps


_NC_CACHE = []


def kernel(positive_prob, predictions, target):
    in_maps = make_in_maps(positive_prob, predictions, target)
    if not _NC_CACHE:
        _NC_CACHE.append(build_nc())
    nc = _NC_CACHE[0]
    res = run_bass_kernel_spmd(nc, in_maps, list(range(NCORES)))
    total = np.float32(0.0)
    for r in res.results:
        total += np.float32(r["out"][0, 0])
    return np.asarray(total, dtype=np.float32)
